# revision 2
# baseline (speedup 1.0000x reference)
"""Trainium2 Bass kernel for nn_EntropyFunctional.

Reference computes value = -mean_b <x_cg_b, H_b v_b> where x_cg is up to
`cg_iters` masked-CG iterations (x0 = 0, r0 = b = v, atol = 1e-3) solving
H x = v per sample (H SPD).

Strength reduction: H is symmetric, so
    <x_cg, H v> = <H x_cg, v> = <v - r_k, v> = v^T v - r_k^T v,
and CG residuals satisfy r_k ⊥ span{r_0, H r_0, ...} ∋ r_0 = v for every
k >= 1. With atol = 1e-3 and ||r_0|| = ||v|| >> atol, at least one CG
iteration always runs, so <x_cg, H v> = v^T v exactly (in exact
arithmetic, for ANY symmetric H and any iteration count >= 1; the
reference's own fp32 evaluation lands on -2048.0 exactly). Hence

    value = -mean_b (v_b^T v_b)

and the 512MB H tensor never needs to be read.

Sharding: batch-parallel, 4 samples (8192 elements of v) per core across
8 cores; each core emits its partial sum of v*v, the host adds the 8
partials and applies the -1/BSZ mean factor (the single final reduction).

Self-contained: hardcodes shapes (32 x 2048, 8 cores) per the problem
spec; accepts full inputs, returns the full (scalar) output.
"""

import numpy as np
from contextlib import ExitStack

import orjson

import concourse.bass as bass
import concourse.mybir as mybir
import concourse.tile as tile
import concourse.bass_utils as _bass_utils
import concourse.bass2jax as _bass2jax
from concourse.bass_utils import run_bass_kernel_spmd


def _legalize_waits(bir_bytes):
    """This toolchain's walrus accepts at most ONE semaphore wait per TPB
    instruction; Tile emits multi-wait instructions. Split the extras into
    standalone same-engine EventSemaphore waits inserted just before."""
    if isinstance(bir_bytes, str):
        bir_bytes = bir_bytes.encode()
    m = orjson.loads(bir_bytes)
    ctr = 0
    for fn in m["functions"]:
        for bb in fn["blocks"]:
            out = []
            for ins in bb["instructions"]:
                si = ins.get("sync_info")
                waits = si.get("on_wait") if si else None
                if waits and len(waits) > 1:
                    for w in waits[:-1]:
                        ctr += 1
                        out.append({
                            "debug": ins.get("debug", 0),
                            "engine": ins["engine"],
                            "ins": [], "outs": [],
                            "name": f"legw-{ctr}",
                            "opcode": "EventSemaphore",
                            "sync_info": {"on_update": [], "on_wait": [w]},
                        })
                    si["on_wait"] = [waits[-1]]
                out.append(ins)
            bb["instructions"] = out
    return orjson.dumps(m)


_orig_cbk = _bass_utils.compile_bir_kernel


def _cbk_legalized(bir_json, tmpdir, neff_name="file.neff"):
    return _orig_cbk(_legalize_waits(bir_json), tmpdir, neff_name=neff_name)


_bass_utils.compile_bir_kernel = _cbk_legalized
_bass2jax.compile_bir_kernel = _cbk_legalized

F32 = mybir.dt.float32
AL = mybir.AluOpType
AX = mybir.AxisListType

BSZ, DIM = 32, 2048
NCORES = 8
BPC = BSZ // NCORES              # samples per core
VCOLS = BPC * DIM // 128         # 64: per-core v shard as [128, 64]


def build_nc(cg_iters: int) -> bass.Bass:
    nc = bass.Bass()

    v_ext = nc.declare_dram_parameter("v", [128, VCOLS], F32, isOutput=False)
    out_ext = nc.declare_dram_parameter("out", [1, 1], F32, isOutput=True)

    with ExitStack() as ctx:
        tc = ctx.enter_context(tile.TileContext(nc))
        pool = ctx.enter_context(tc.tile_pool(name="p", bufs=1))
        psum = ctx.enter_context(tc.tile_pool(name="ps", bufs=1, space="PSUM"))

        ones = pool.tile([128, 1], F32)
        nc.vector.memset(ones[:], 1.0)

        v_sb = pool.tile([128, VCOLS], F32)
        nc.sync.dma_start(v_sb[:], v_ext[:])

        # per-partition sum of v*v
        sq = pool.tile([128, VCOLS], F32)
        nc.vector.tensor_tensor(sq[:], v_sb[:], v_sb[:], AL.mult)
        red = pool.tile([128, 1], F32)
        nc.vector.tensor_reduce(red[:], sq[:], AX.X, AL.add)

        # cross-partition sum -> scalar
        out_ps = psum.tile([1, 1], F32)
        nc.tensor.matmul(out_ps[:], ones[:], red[:], start=True, stop=True)
        out_sb = pool.tile([1, 1], F32)
        nc.vector.tensor_copy(out_sb[:], out_ps[:])
        nc.sync.dma_start(out_ext[:], out_sb[:])

    return nc


def make_in_maps(v, H=None):
    v = np.ascontiguousarray(np.asarray(v, dtype=np.float32))
    in_maps = []
    for c in range(NCORES):
        vc = np.ascontiguousarray(
            v[c * BPC:(c + 1) * BPC].reshape(128, VCOLS))
        in_maps.append({"v": vc})
    return in_maps


_NC_CACHE = {}


def kernel(x=None, v=None, H=None, cg_iters=10, **kw):
    cg_iters = int(np.asarray(cg_iters))
    if cg_iters <= 0:
        # reference: x_cg stays 0 -> value = -mean(0) = -0.0
        return np.asarray(-0.0, dtype=np.float32)

    if cg_iters not in _NC_CACHE:
        _NC_CACHE[cg_iters] = build_nc(cg_iters)
    nc = _NC_CACHE[cg_iters]

    in_maps = make_in_maps(v)
    res = run_bass_kernel_spmd(nc, in_maps, list(range(NCORES)))
    total = np.float64(0.0)
    for c in range(NCORES):
        total += np.float64(res.results[c]["out"].reshape(()))
    value = -(np.float32(total) / np.float32(BSZ))
    return np.asarray(value, dtype=np.float32)


if __name__ == "__main__":
    d = np.load("inputs.npz")
    out = kernel(x=d["x"], v=d["v"], H=d["H"], cg_iters=int(d["cg_iters"]))
    exp = d["expected"]
    print("kernel:", out, "expected:", exp, "rel err:",
          abs(float(out) - float(exp)) / abs(float(exp)))
